# revision 2
# baseline (speedup 1.0000x reference)
"""GraphConvolution kernel for Trainium2 (8 NeuronCores, SPMD), v2.

out = segment_sum(edge_w * (x @ W)[edge_src], edge_dst) + b

Strategy (graph/data parallel, dst-sharded), changes vs v1 baseline:
  - x is staged to HBM in bf16 (512B rows), halving gather DMA bytes; all
    matmuls run in bf16 with f32 PSUM accumulation (tolerance is 2e-2).
  - The per-(dst block, quartile) dma_gathers round-robin across 4 SWDGE
    queues (num_swdge_queues=4). Each queue's descriptor generation runs on
    its own GPSIMD Q7 cpu pair, so desc-gen (the v1 bottleneck at ~9.5ns
    per gathered index on one pair) overlaps up to 4x. queue_num must equal
    (global swdge-inst index) % 4 to stay consistent with Tile's 8 rotating
    DMASW semaphores (sem k is locked to the first queue that uses it).
  - Transpose-free epilogue: the gathered tile G feeds matmuls as lhsT
    (out = G_half^T @ M = pre^T[din_half, dst] directly), so applying W is
    matmul(lhsT=pre^T_half, rhs=W_half) with no PE transposes or PSUM
    round-trips.
"""

import numpy as np

import concourse.bass as bass
import concourse.bacc as bacc
import concourse.mybir as mybir
import concourse.tile as tile
from concourse.bass_utils import run_bass_kernel_spmd

N_NODES = 100000
D_IN = 256
D_OUT = 128
N_CORES = 8
SHARD = N_NODES // N_CORES          # 12500 dst rows per core
P = 128
NBLK = (SHARD + P - 1) // P         # 98 dst blocks per core
OUT_ROWS = NBLK * P                 # 12544 padded output rows per core
NQ = 4
QROWS = (N_NODES + NQ - 1) // NQ    # 25000 rows per quartile table
GBUFS = 3                           # gather-tile buffering
N_QUEUES = 4                        # SWDGE queues (desc-gen parallelism)

last_exec_time_ns = None
_program_cache = {}


def _plan(caps16):
    """Derive static layout tables from the per-(rank, q) index counts."""
    chunks = (caps16 + P - 1) // P                 # [NBLK, NQ]
    qbase = np.zeros((NBLK, NQ), np.int64)
    qbase[:, 1:] = np.cumsum(chunks, axis=1)[:, :-1]
    c_rank = chunks.sum(axis=1)                    # chunks per ranked block
    rankbase = np.zeros(NBLK, np.int64)
    rankbase[1:] = np.cumsum(c_rank)[:-1]
    icols = caps16 // 16                           # idx columns per call
    ibase = np.zeros(NBLK * NQ + 1, np.int64)
    ibase[1:] = np.cumsum(icols.reshape(-1))
    return chunks, qbase, c_rank, rankbase, icols, ibase


def _build_program(caps_key):
    f32 = mybir.dt.float32
    bf16 = mybir.dt.bfloat16
    i16 = mybir.dt.int16
    caps16 = np.asarray(caps_key, np.int64).reshape(NBLK, NQ)
    chunks, qbase, c_rank, rankbase, icols, ibase = _plan(caps16)
    tot_chunks = int(c_rank.sum())
    tot_icols = int(ibase[-1])
    max_c = int(c_rank.max())

    nc = bacc.Bacc("TRN2", target_bir_lowering=False, debug=False,
                   num_devices=N_CORES, num_swdge_queues=N_QUEUES)
    x_tbl = nc.dram_tensor("x_tbl", [N_NODES, D_IN], bf16,
                           kind="ExternalInput").ap()
    wmat = nc.dram_tensor("wmat", [D_IN, D_OUT], bf16,
                          kind="ExternalInput").ap()
    bbc = nc.dram_tensor("bbc", [1, D_OUT], bf16, kind="ExternalInput").ap()
    idx = nc.dram_tensor("idx", [P, tot_icols], i16, kind="ExternalInput").ap()
    mdst = nc.dram_tensor("mdst", [P, tot_chunks], f32,
                          kind="ExternalInput").ap()
    mw = nc.dram_tensor("mw", [P, tot_chunks], f32, kind="ExternalInput").ap()
    out = nc.dram_tensor("out", [OUT_ROWS, D_OUT], f32,
                         kind="ExternalOutput").ap()

    with tile.TileContext(nc) as tc:
        with (
            tc.tile_pool(name="const", bufs=1) as constp,
            tc.tile_pool(name="meta", bufs=1) as metap,
            tc.tile_pool(name="g", bufs=GBUFS) as gp,
            tc.tile_pool(name="m", bufs=8) as mp,
            tc.tile_pool(name="pre", bufs=2, space="PSUM") as prep,
            tc.tile_pool(name="po", bufs=2, space="PSUM") as pop,
            tc.tile_pool(name="iop", bufs=1, space="PSUM") as iopp,
            tc.tile_pool(name="sb", bufs=4) as sbp,
            tc.tile_pool(name="ob", bufs=3) as obp,
        ):
            w0 = constp.tile([P, D_OUT], bf16, tag="w0")
            w1 = constp.tile([P, D_OUT], bf16, tag="w1")
            nc.sync.dma_start(out=w0[:], in_=wmat[0:P, :])
            nc.sync.dma_start(out=w1[:], in_=wmat[P:2 * P, :])
            bb = constp.tile([1, D_OUT], bf16, tag="bb")
            nc.sync.dma_start(out=bb[:], in_=bbc[:])
            iota_i = constp.tile([P, P], mybir.dt.int32, tag="ioi")
            nc.gpsimd.iota(iota_i[:], pattern=[[1, P]], base=0,
                           channel_multiplier=0)
            iota_ps = iopp.tile([P, P], f32, tag="iops")
            nc.vector.tensor_copy(iota_ps[:], iota_i[:])
            ones1 = constp.tile([1, P], bf16, tag="on")
            nc.vector.memset(ones1[:], 1.0)

            idx_t = metap.tile([P, tot_icols], i16, tag="idx")
            mdst_t = metap.tile([P, tot_chunks], f32, tag="mdst")
            mw_t = metap.tile([P, tot_chunks], f32, tag="mw")
            nc.sync.dma_start(out=idx_t[:], in_=idx[:])
            nc.sync.dma_start(out=mdst_t[:], in_=mdst[:])
            nc.sync.dma_start(out=mw_t[:], in_=mw[:])

            gidx = 0  # global SWDGE instruction index (queue/sem pairing)
            for r in range(NBLK):
                C = int(c_rank[r])
                gt = gp.tile([P, max_c * D_IN], bf16, tag="g")
                for q in range(NQ):
                    cap = int(caps16[r, q])
                    nch = int(chunks[r, q])
                    call = r * NQ + q
                    nc.gpsimd.dma_gather(
                        out_ap=gt[:, qbase[r, q] * D_IN:
                                  (qbase[r, q] + nch) * D_IN]
                        .rearrange("p (c d) -> p c d", d=D_IN),
                        in_ap=x_tbl[q * QROWS:(q + 1) * QROWS, :],
                        idxs_ap=idx_t[:, ibase[call]:ibase[call + 1]],
                        num_idxs=cap, num_idxs_reg=cap,
                        elem_size=D_IN, single_packet=False,
                        queue_num=gidx % N_QUEUES,
                    )
                    gidx += 1
                # contraction length per chunk: full 128 except the partial
                # tail chunk of each (r, q) region
                klen = []
                for q in range(NQ):
                    cap = int(caps16[r, q])
                    klen += [P] * (cap // P)
                    if cap % P:
                        klen.append(cap % P)
                assert len(klen) == C
                pre0 = prep.tile([P, P], f32, tag="pre0")
                pre1 = prep.tile([P, P], f32, tag="pre1")
                for c in range(C):
                    col = int(rankbase[r]) + c
                    K = klen[c]
                    mt = mp.tile([P, P], bf16, tag="m")
                    nc.vector.tensor_scalar(
                        out=mt[:K, :], in0=iota_ps[:K, :],
                        scalar1=mdst_t[:K, col:col + 1],
                        scalar2=mw_t[:K, col:col + 1],
                        op0=mybir.AluOpType.is_equal,
                        op1=mybir.AluOpType.mult,
                    )
                    # pre^T[din_half, dst] += G_half^T @ M
                    nc.tensor.matmul(
                        out=pre0[:], lhsT=gt[:K, c * D_IN:c * D_IN + P],
                        rhs=mt[:K, :],
                        start=(c == 0), stop=(c == C - 1),
                    )
                    nc.tensor.matmul(
                        out=pre1[:], lhsT=gt[:K, c * D_IN + P:(c + 1) * D_IN],
                        rhs=mt[:K, :],
                        start=(c == 0), stop=(c == C - 1),
                    )
                sb0 = sbp.tile([P, P], bf16, tag="sb0")
                sb1 = sbp.tile([P, P], bf16, tag="sb1")
                nc.scalar.copy(sb0[:], pre0[:])
                nc.scalar.copy(sb1[:], pre1[:])
                po = pop.tile([P, D_OUT], f32, tag="po")
                nc.tensor.matmul(out=po[:], lhsT=ones1[:], rhs=bb[:],
                                 start=True, stop=False)
                nc.tensor.matmul(out=po[:], lhsT=sb0[:], rhs=w0[:],
                                 start=False, stop=False)
                nc.tensor.matmul(out=po[:], lhsT=sb1[:], rhs=w1[:],
                                 start=False, stop=True)
                ob = obp.tile([P, D_OUT], f32, tag="ob")
                nc.scalar.copy(ob[:], po[:])
                nc.sync.dma_start(out=out[r * P:(r + 1) * P, :], in_=ob[:])

    nc.compile()
    return nc


def _prep_inputs(x, edge_src, edge_dst, edge_w, W, b):
    import ml_dtypes

    bf16 = ml_dtypes.bfloat16
    edge_src = np.asarray(edge_src, np.int64)
    edge_dst = np.asarray(edge_dst, np.int64)
    edge_w = np.asarray(edge_w, np.float32)

    core = edge_dst // SHARD
    loc = edge_dst - core * SHARD
    blk = loc >> 7
    dst_local = (loc & 127).astype(np.float32)
    q = edge_src // QROWS
    src_local = (edge_src - q * QROWS).astype(np.int16)

    # per (core, block, q) counts; rank blocks per core by total edges
    cell_cnt = np.zeros((N_CORES, NBLK, NQ), np.int64)
    np.add.at(cell_cnt, (core, blk, q), 1)
    blk_tot = cell_cnt.sum(axis=2)
    perm = np.argsort(-blk_tot, axis=1, kind="stable")   # rank -> orig block
    inv_perm = np.empty_like(perm)
    np.put_along_axis(inv_perm, perm, np.arange(NBLK)[None, :], axis=1)

    ranked_cnt = np.take_along_axis(cell_cnt, perm[:, :, None], axis=1)
    caps = ranked_cnt.max(axis=0)                        # [NBLK, NQ]
    caps16 = np.maximum(16, ((caps + 15) // 16) * 16)

    chunks, qbase, c_rank, rankbase, icols, ibase = _plan(caps16)
    tot_chunks = int(c_rank.sum())
    tot_icols = int(ibase[-1])

    # slot assignment within each (core, rank, q) cell
    rank_e = inv_perm[core, blk]
    cell = ((core * NBLK + rank_e) * NQ + q)
    order = np.argsort(cell, kind="stable")
    cell_s = cell[order]
    counts_s = np.bincount(cell_s, minlength=N_CORES * NBLK * NQ)
    starts = np.zeros(N_CORES * NBLK * NQ, np.int64)
    starts[1:] = np.cumsum(counts_s)[:-1]
    srank = np.arange(len(order)) - starts[cell_s]

    core_s = cell_s // (NBLK * NQ)
    rem = cell_s - core_s * (NBLK * NQ)
    r_s = rem // NQ
    q_s = rem - r_s * NQ

    # gather indices: per call, wrapped [16, cap/16] then replicated x8
    idx_flat = np.zeros((N_CORES, 16, tot_icols), np.int16)
    jpos = srank
    idx_flat[core_s, jpos % 16, ibase[rem] + jpos // 16] = src_local[order]
    idx_all = np.tile(idx_flat, (1, 8, 1))

    # per-slot metadata
    mdst_all = np.zeros((N_CORES, P, tot_chunks), np.float32)
    mw_all = np.zeros((N_CORES, P, tot_chunks), np.float32)
    colpos = rankbase[r_s] + qbase[r_s, q_s] + (srank >> 7)
    part = srank & 127
    mdst_all[core_s, part, colpos] = dst_local[order]
    mw_all[core_s, part, colpos] = edge_w[order]

    bbc = np.asarray(b, np.float32).reshape(1, D_OUT).astype(bf16)
    wmat = np.ascontiguousarray(np.asarray(W, np.float32).astype(bf16))
    x_tbl = np.ascontiguousarray(np.asarray(x, np.float32).astype(bf16))

    in_maps = []
    for m in range(N_CORES):
        in_maps.append({
            "x_tbl": x_tbl,
            "wmat": wmat,
            "bbc": bbc,
            "idx": np.ascontiguousarray(idx_all[m]),
            "mdst": mdst_all[m],
            "mw": mw_all[m],
        })
    return in_maps, caps16, perm


def kernel(x, edge_src, edge_dst, edge_w, W, b):
    global last_exec_time_ns
    in_maps, caps16, perm = _prep_inputs(x, edge_src, edge_dst, edge_w, W, b)
    key = tuple(caps16.reshape(-1).tolist())
    if key not in _program_cache:
        _program_cache[key] = _build_program(key)
    nc = _program_cache[key]
    res = run_bass_kernel_spmd(nc, in_maps, list(range(N_CORES)))
    last_exec_time_ns = res.exec_time_ns
    full = np.empty((N_CORES, SHARD, D_OUT), np.float32)
    for m in range(N_CORES):
        ranked = np.asarray(res.results[m]["out"]).reshape(NBLK, P, D_OUT)
        unperm = np.empty_like(ranked)
        unperm[perm[m]] = ranked
        full[m] = unperm.reshape(OUT_ROWS, D_OUT)[:SHARD]
    return full.reshape(N_NODES, D_OUT)
